# revision 1
# baseline (speedup 1.0000x reference)
"""BitLinear (ternary-quantized linear) Trainium2 kernel.

Computes: out = x @ dequant(weight).T where dequant is per-group(128)
AbsMean ternary quantization (w_q in {-1,0,+1} times per-group scale).

Strategy (8 NeuronCores, column-parallel / tensor-parallel):
  - weight [O=11008, K=4096] sharded by rows across 8 cores (1376 each).
  - x [T=8192, K] replicated, host-packed so each t-tile load is
    contiguous; streamed in half-tiles and cast to fp16 on ACT.
  - On-chip dequant per o-tile (128 rows), engine-balanced:
      DVE:    per-group abs-sum reduce, thresholds, compare |w|>s/2,
              q = c*sign (packed fp16)
      ACT:    |w| (Abs) and sign(w) (Sign), transpose evacuations
      GpSimd: w_eff = q * fp16(s) broadcast mult, w DMA ring
      PE:     [o,k]->[k,o] transposes (identity matmul)
    The first two o-tiles are processed in ko-halves to cut latency.
  - Resident weight: 5 ko-major fp16 tiles of 256/256/256/256/352
    columns.  Separate tiles give 2-o-tile dependency granularity while
    keeping matmul rhs streams contiguous (256+ cols hides LDWEIGHTS).
  - PSUM: two 256-wide tiles accumulate into one 2KB bank; the second
    uses start=False at ko==0 and lands on the pending-zeroed bank.
  - Schedule: early t-tiles process a growing prefix of the 5 weight
    tiles as dequant completes; suffixes are caught up by extra visits
    interleaved into the steady phase (all-column visits).
  - Per-core output [T, 1376]; host concatenates along O.
"""

import os

import numpy as np

import concourse.bass as bass
import concourse.mybir as mybir
import concourse.tile as tile
from concourse import bacc
from concourse.bass_utils import run_bass_kernel_spmd
from concourse.masks import make_identity

P = 128
GROUP = 128
EPS = 1e-8

FULL_B, FULL_S, FULL_K, FULL_O = 4, 2048, 4096, 11008
N_CORES = 8

LAST_RESULT = None  # BassKernelResults of the most recent run (for test.py)

# Weight-tile column widths (must each be >=256 except via warmup use).
WTILE_COLS = [256, 256, 256, 256, 352]
# psum pool index for each weight tile (two 256s share one bank).
WTILE_POOL = [0, 0, 1, 1, 2]
# Warmup plan: number of weight tiles (prefix) each early t-tile covers.
WARM_UNITS = [1, 1, 1, 1, 2, 2, 2, 3, 3, 3, 4, 4, 4, 4]
CATCHUP_EVERY = 3


def build_program(K, T, O_SHARD, mm_dt=mybir.dt.float16):
    assert K % GROUP == 0 and T % P == 0
    KO = K // GROUP
    KH = KO // 2
    TB = 8
    assert KO % TB == 0
    n_ttiles = T // P
    o_tiles = [(o0, min(P, O_SHARD - o0)) for o0 in range(0, O_SHARD, P)]
    n_ot = len(o_tiles)
    assert sum(WTILE_COLS) == O_SHARD
    wt_off = [sum(WTILE_COLS[:i]) for i in range(len(WTILE_COLS))]

    nc = bacc.Bacc("TRN2", target_bir_lowering=False, debug=False)
    xt = nc.dram_tensor("xt", [T, K], mybir.dt.float32, kind="ExternalInput").ap()
    w = nc.dram_tensor(
        "w", [O_SHARD, K], mybir.dt.float32, kind="ExternalInput"
    ).ap()
    out = nc.dram_tensor(
        "out", [T, O_SHARD], mybir.dt.float32, kind="ExternalOutput"
    ).ap()

    with tile.TileContext(nc) as tc:
        with (
            tc.tile_pool(name="wres", bufs=1) as wres,
            tc.tile_pool(name="const", bufs=1) as constp,
            tc.tile_pool(name="deq32", bufs=2) as deq32,
            tc.tile_pool(name="deq16", bufs=4) as deq16,
            tc.tile_pool(name="tiny", bufs=2) as tiny,
            tc.tile_pool(name="xf32", bufs=3) as xf32,
            tc.tile_pool(name="xin", bufs=2) as xin,
            tc.tile_pool(name="outp", bufs=2) as outp,
            tc.tile_pool(name="ps_tp", bufs=2, space="PSUM") as ps_tp,
            tc.tile_pool(name="ps_a", bufs=2, space="PSUM") as ps_a,
            tc.tile_pool(name="ps_b", bufs=2, space="PSUM") as ps_b,
            tc.tile_pool(name="ps_c", bufs=2, space="PSUM") as ps_c,
        ):
            ps_pools = [ps_a, ps_b, ps_c]
            # Resident dequantized transposed weight, ko-major per wtile:
            # wbt[wi][p, ko, col] with contiguous columns per ko (fast rhs).
            wbt = [
                wres.tile([P, KO, csz], mm_dt, tag=f"wbt{wi}", name=f"wbt{wi}")
                for wi, csz in enumerate(WTILE_COLS)
            ]
            ident = constp.tile([P, P], mm_dt)
            make_identity(nc, ident)

            # ------------- dequant of one o-tile (optionally split) -------
            def emit_deq(i, splits=1):
                o0, osz = o_tiles[i]
                wi = next(
                    j for j, c0 in enumerate(wt_off)
                    if c0 <= o0 < c0 + WTILE_COLS[j]
                )
                lo = o0 - wt_off[wi]
                wt = deq32.tile([P, KO, GROUP], mybir.dt.float32, tag="wt",
                                name="wt")
                sgn = deq16.tile([P, KO, GROUP], mm_dt, tag="dq", name="sgn")
                c = deq16.tile([P, KO, GROUP], mm_dt, tag="dq", name="c")
                cs = deq16.tile([P, KO, GROUP], mm_dt, tag="dq", name="cs")
                wb = deq16.tile([P, KO, GROUP], mm_dt, tag="dq", name="wb")
                sums = tiny.tile([P, KO], mybir.dt.float32, tag="sums")
                tpos = tiny.tile([P, KO], mybir.dt.float32, tag="tpos")
                s16 = tiny.tile([P, KO], mm_dt, tag="s16")
                wsrc = w[o0 : o0 + osz].rearrange("o (ko k) -> o ko k", k=GROUP)
                step = KO // splits
                for h in range(splits):
                    ka, kb_ = h * step, (h + 1) * step
                    kr = kb_ - ka
                    nc.sync.dma_start(wt[:osz, ka:kb_], wsrc[:, ka:kb_])
                    nc.vector.tensor_reduce(
                        sums[:osz, ka:kb_], wt[:osz, ka:kb_],
                        axis=mybir.AxisListType.X, op=mybir.AluOpType.add,
                        apply_absolute_value=True,
                    )
                    nc.vector.tensor_scalar(
                        tpos[:osz, ka:kb_], sums[:osz, ka:kb_],
                        0.5 / GROUP, 0.5 * EPS,
                        mybir.AluOpType.mult, mybir.AluOpType.max,
                    )
                    nc.vector.tensor_scalar(
                        s16[:osz, ka:kb_], sums[:osz, ka:kb_],
                        1.0 / GROUP, EPS,
                        mybir.AluOpType.mult, mybir.AluOpType.max,
                    )
                    nc.scalar.activation(
                        sgn[:osz, ka:kb_], wt[:osz, ka:kb_],
                        mybir.ActivationFunctionType.Sign,
                    )
                    nc.scalar.activation(
                        wt[:osz, ka:kb_], wt[:osz, ka:kb_],
                        mybir.ActivationFunctionType.Abs,
                    )
                    nc.vector.tensor_tensor(
                        c[:osz, ka:kb_], wt[:osz, ka:kb_],
                        tpos[:osz, ka:kb_, None].to_broadcast((osz, kr, GROUP)),
                        mybir.AluOpType.is_gt,
                    )
                    nc.vector.tensor_tensor(
                        cs[:osz, ka:kb_], c[:osz, ka:kb_], sgn[:osz, ka:kb_],
                        mybir.AluOpType.mult,
                    )
                    nc.gpsimd.tensor_tensor(
                        wb[:osz, ka:kb_], cs[:osz, ka:kb_],
                        s16[:osz, ka:kb_, None].to_broadcast((osz, kr, GROUP)),
                        mybir.AluOpType.mult,
                    )
                    for kb in range(ka, kb_, TB):
                        ps = ps_tp.tile([P, TB, P], mm_dt, tag="tp")
                        for j in range(TB):
                            nc.tensor.transpose(
                                ps[:, j, :osz], wb[:osz, kb + j, :],
                                ident[:osz, :osz],
                            )
                        nc.scalar.copy(
                            wbt[wi][:, kb : kb + TB, lo : lo + osz],
                            ps[:, :, :osz],
                        )

            # ------------- one t-tile visit over column range -------------
            xt_r = xt.rearrange("(tt p) (ko t) -> tt p ko t", p=P, t=P)

            def emit_tile(tt, col_lo, col_hi, out_ring=None):
                xb = xin.tile([P, KO, P], mm_dt, tag="xb")
                for h in range(2):
                    ka, kb_ = h * KH, (h + 1) * KH
                    xf = xf32.tile([P, KH, P], mybir.dt.float32, tag="xf")
                    nc.sync.dma_start(xf, xt_r[tt][:, ka:kb_])
                    nc.scalar.copy(xb[:, ka:kb_], xf)
                t0 = tt * P
                ot = outp.tile([P, O_SHARD], mybir.dt.float32, tag="ot")
                spans = []  # (wi, a, b) local col ranges per weight tile
                for wi, csz in enumerate(WTILE_COLS):
                    a = max(col_lo - wt_off[wi], 0)
                    bnd = min(csz, col_hi - wt_off[wi])
                    if a < bnd:
                        spans.append((wi, a, bnd))
                pss = {}
                first_in_pool = {}
                for wi, a, bnd in spans:
                    pi = WTILE_POOL[wi]
                    if pi not in pss:
                        pss[pi] = ps_pools[pi].tile(
                            [P, 512], mybir.dt.float32, tag=f"mm{pi}",
                            name=f"mm{pi}",
                        )
                        first_in_pool[pi] = wi
                for ko in range(KO):
                    for wi, a, bnd in spans:
                        pi = WTILE_POOL[wi]
                        poff = wt_off[wi] - wt_off[
                            next(j for j in range(len(WTILE_COLS))
                                 if WTILE_POOL[j] == pi)
                        ]
                        nc.tensor.matmul(
                            pss[pi][:, poff + a : poff + bnd],
                            lhsT=xb[:, ko, :],
                            rhs=wbt[wi][:, ko, a:bnd],
                            start=(ko == 0 and first_in_pool[pi] == wi),
                            stop=(ko == KO - 1),
                        )
                for wi, a, bnd in spans:
                    pi = WTILE_POOL[wi]
                    poff = wt_off[wi] - wt_off[
                        next(j for j in range(len(WTILE_COLS))
                             if WTILE_POOL[j] == pi)
                    ]
                    ga = wt_off[wi] + a
                    nc.scalar.copy(
                        ot[:, ga : ga + (bnd - a)],
                        pss[pi][:, poff + a : poff + bnd],
                    )
                (out_ring or nc.sync).dma_start(
                    out[t0 : t0 + P, col_lo:col_hi], ot[:, col_lo:col_hi]
                )

            # ------------- emission schedule -------------
            deq_done = 0

            def ensure_deq(target):
                nonlocal deq_done
                while deq_done < min(target, n_ot):
                    emit_deq(deq_done, splits=2 if deq_done < 2 else 1)
                    deq_done += 1

            for j, units in enumerate(WARM_UNITS):
                ensure_deq(max(2 * units, j + 2))
                emit_tile(j, 0, wt_off[units - 1] + WTILE_COLS[units - 1])
            ensure_deq(n_ot)

            catchups = [
                (j, wt_off[u - 1] + WTILE_COLS[u - 1])
                for j, u in enumerate(WARM_UNITS)
                if wt_off[u - 1] + WTILE_COLS[u - 1] < O_SHARD
            ]
            steady = list(range(len(WARM_UNITS), n_ttiles))
            ci = 0
            for k, tt in enumerate(steady):
                emit_tile(tt, 0, O_SHARD)
                if (k + 1) % CATCHUP_EVERY == 0 and ci < len(catchups):
                    jj, start_col = catchups[ci]
                    emit_tile(jj, start_col, O_SHARD)
                    ci += 1
            while ci < len(catchups):
                jj, start_col = catchups[ci]
                emit_tile(jj, start_col, O_SHARD)
                ci += 1

    nc.compile()
    return nc


def _run(nc, in_maps, trace=False):
    global LAST_RESULT
    res = run_bass_kernel_spmd(
        nc, in_maps, core_ids=list(range(len(in_maps))), trace=trace
    )
    LAST_RESULT = res
    return res


def pack_x(x2d):
    """[T, K] -> packed layout: H[tt*P+p, ko*G+t] = x2d[tt*P+t, ko*G+p]."""
    T, K = x2d.shape
    x4 = x2d.reshape(T // P, P, K // GROUP, GROUP)  # [tt, t, ko, p]
    return np.ascontiguousarray(x4.transpose(0, 3, 2, 1).reshape(T, K))


def kernel(x, weight):
    T = FULL_B * FULL_S
    K = FULL_K
    OS = FULL_O // N_CORES  # 1376
    x2d = pack_x(np.asarray(x, dtype=np.float32).reshape(T, K))
    w = np.asarray(weight, dtype=np.float32)

    nc = build_program(K, T, OS)
    in_maps = [
        {"xt": x2d, "w": np.ascontiguousarray(w[c * OS : (c + 1) * OS])}
        for c in range(N_CORES)
    ]
    trace = bool(os.environ.get("BASS_TRACE"))
    res = _run(nc, in_maps, trace=trace)
    full = np.concatenate(
        [res.results[c]["out"] for c in range(N_CORES)], axis=1
    )
    return np.ascontiguousarray(full.reshape(FULL_B, FULL_S, FULL_O))

